# revision 8
# baseline (speedup 1.0000x reference)
"""Causal self-attention (B=2, T=2048, C=1024, NH=16, HS=64) on 8 TRN2 NeuronCores.

Sharding: core c -> batch b = c//4, head-group g = c%4 (4 heads per core).
Each core computes qkv projection for its 768 W columns + causal attention for
its 4 heads; host concatenates the per-core [T, 256] outputs.

Layout strategy per core:
  - x[b] is PE-transposed to xT [C_sub=128, 8, T] once.
  - q, k are produced transposed ([d, t], head-pairs packed 2x64 on partitions)
    so QK^T runs as scoresT[k, q] = kT.T @ qT with row-tiled (tile_position)
    pairs; softmax reduction dim lands on partitions, which the PV matmul
    contracts directly (no per-block transposes of the attention weights).
  - v is produced natural ([t, d]) with a ones-column appended, so the PV
    matmul emits [65, q]: rows 0:64 = head output^T, row 64 = softmax sums.
  - [65, 512] output blocks are PE-transposed back to [q, 65]; a per-partition
    reciprocal of col 64 normalizes rows 0:64.
Matmuls run as float32r (fp32 bits, single-pass PE streaming; ~1e-3 worst-case
relative error measured on HW) except the exact PE transposes.
"""
import sys

sys.path.insert(0, "/opt/trn_rl_repo")

import numpy as np

import concourse.bass as bass
import concourse.tile as tile
from concourse import bacc, mybir
from concourse import bass_utils
from concourse.bass import ds, ts
from concourse.masks import make_identity

B, T, C, NH, HS = 2, 2048, 1024, 16, 64
NCORES = 8
HPC = NH // 4  # heads per core = 4
GCOLS = HPC * HS  # 256 W columns per section per core
F32 = mybir.dt.float32
F32R = mybir.dt.float32r
AF = mybir.ActivationFunctionType
ALU = mybir.AluOpType

USE_F32R = True


DT_MM = F32R if USE_F32R else F32


def _r(ap):
    return ap


def _emit(tc, nc, xb, w, bvec, out_d):
    P = 128
    KS = C // P  # 8 contraction subtiles
    NTT = T // P  # 16 t-tiles
    QCS = (0, 512, 1024, 1536)

    import contextlib
    _stack = contextlib.ExitStack()
    singles = _stack.enter_context(tc.tile_pool(name="singles", bufs=1))

    ident = singles.tile([P, P], F32)
    make_identity(nc, ident[:])

    # tri[k, m] = 1 if m >= k else 0  (keep upper-incl-diag of the 128x128
    # diagonal block in scoresT layout)
    tri = singles.tile([P, P], F32)
    nc.vector.memset(tri[:], 1.0)
    nc.gpsimd.affine_select(
        out=tri[:], in_=tri[:], compare_op=ALU.is_ge, fill=0.0,
        base=0, pattern=[[1, P]], channel_multiplier=-1,
    )

    # per-partition bias tiles for the transposed q/k layouts
    bq = [singles.tile([P, 1], F32, tag=f"bq{p}", name=f"bq{p}") for p in range(2)]
    bk = [singles.tile([P, 1], F32, tag=f"bk{p}", name=f"bk{p}") for p in range(2)]
    for p in range(2):
        nc.sync.dma_start(bq[p][:], bvec[ds(p * P, P)].rearrange("(p o) -> p o", o=1))
        nc.sync.dma_start(bk[p][:], bvec[ds(GCOLS + p * P, P)].rearrange("(p o) -> p o", o=1))
    bv = singles.tile([P, HPC, HS], F32)
    _bv_src = bvec[ds(2 * GCOLS, GCOLS)].rearrange("(h d) -> h d", h=HPC)
    nc.sync.dma_start(bv[:], bass.AP(tensor=_bv_src.tensor, offset=_bv_src.offset,
                                     ap=[[0, P], *_bv_src.ap]))

    wsb = singles.tile([P, KS, 3 * GCOLS], DT_MM)
    nc.sync.dma_start(wsb[:], w.rearrange("(ko ki) n -> ki ko n", ki=P))

    xT = singles.tile([P, KS, T], DT_MM)
    qT = singles.tile([P, 2, T], DT_MM)
    kT = singles.tile([P, 2, T], DT_MM)
    vA = singles.tile([P, NTT, HPC, HS + 1], DT_MM)
    ones64 = singles.tile([P, NTT * HPC], F32)
    nc.vector.memset(ones64[:], 1.0)
    nc.vector.tensor_copy(
        vA[:, :, :, HS:HS + 1].rearrange("p a b o -> p (a b o)"), ones64[:]
    )

    # ---- phase 1: transpose x[b] -> xT -------------------------------------
    with (
        tc.tile_pool(name="xin", bufs=6) as xin,
        tc.tile_pool(name="ps_tr", bufs=2, space="PSUM") as ps_tr,
    ):
        for tg in range(NTT // 4):
            xtiles = []
            for i in range(4):
                xt = xin.tile([P, C], F32, tag="xt")
                nc.sync.dma_start(xt[:], xb[ds((tg * 4 + i) * P, P), :])
                xtiles.append(xt)
            for ko in range(KS):
                pt = ps_tr.tile([P, 4, P], F32, tag="pt")
                for i in range(4):
                    nc.tensor.matmul(
                        pt[:, i, :], xtiles[i][:, ts(ko, P)], ident[:],
                        is_transpose=True, start=(i == 0), stop=(i == 3),
                    )
                dst = xT[:, ko, ts(tg, 512)]
                if ko % 2 == 0:
                    nc.vector.tensor_copy(dst, pt[:].rearrange("p a b -> p (a b)"))
                else:
                    nc.scalar.copy(dst, pt[:].rearrange("p a b -> p (a b)"))

    # ---- phase 2: qkv projection -------------------------------------------
    with tc.tile_pool(name="ps_qkv", bufs=4, space="PSUM") as ps_qkv:
        # qT / kT: [d-pair 128, t] = W_slice.T @ xT
        for sec, dstT, btiles in ((0, qT, bq), (GCOLS, kT, bk)):
            for pair in range(2):
                for tch in range(T // 512):
                    pq = ps_qkv.tile([P, 512], F32, tag="pq")
                    for k in range(KS):
                        nc.tensor.matmul(
                            pq[:],
                            _r(wsb[:, k, ds(sec + pair * P, P)]),
                            _r(xT[:, k, ts(tch, 512)]),
                            start=(k == 0), stop=(k == KS - 1),
                        )
                    nc.vector.tensor_scalar_add(
                        dstT[:, pair, ts(tch, 512)], pq[:], btiles[pair][:]
                    )
        # v: [t 128, 4*64] = xT_chunk.T @ Wv
        for tt in range(NTT):
            pv = ps_qkv.tile([P, GCOLS], F32, tag="pv")
            for k in range(KS):
                nc.tensor.matmul(
                    pv[:],
                    _r(xT[:, k, ts(tt, P)]),
                    _r(wsb[:, k, ds(2 * GCOLS, GCOLS)]),
                    start=(k == 0), stop=(k == KS - 1),
                )
            nc.vector.tensor_tensor(
                vA[:, tt, :, 0:HS],
                pv[:].rearrange("p (h d) -> p h d", h=HPC),
                bv[:],
                ALU.add,
            )

    # ---- phase 3: attention ------------------------------------------------
    with (
        tc.tile_pool(name="ps_sc", bufs=3, space="PSUM") as ps_sc,
        tc.tile_pool(name="ps_pv", bufs=3, space="PSUM") as ps_pv,
        tc.tile_pool(name="ps_nrm", bufs=2, space="PSUM") as ps_nrm,
        tc.tile_pool(name="wei", bufs=6) as weip,
        tc.tile_pool(name="otp", bufs=2) as otp,
        tc.tile_pool(name="fin", bufs=8) as fin,
    ):
        for pair in range(2):
            for qc in QCS:
                jmax = min(NTT - 1, qc // P + 3)
                pvh = [ps_pv.tile([HS + 1, 512], F32, tag="pvps", name="pvps") for _ in range(2)]
                for j in range(jmax + 1):
                    diag = (j * P) // 512 * 512 == qc
                    o = j * P - qc if diag else 0
                    sc = []
                    for hh in range(2):
                        s = ps_sc.tile([P, 512], F32, tag="scps")
                        nc.tensor.matmul(
                            s[:, o:512],
                            _r(kT[ds(hh * HS, HS), pair, ts(j, P)]),
                            _r(qT[ds(hh * HS, HS), pair, ds(qc + o, 512 - o)]),
                            start=True, stop=True,
                            tile_position=(hh * HS, 0),
                        )
                        sc.append(s)
                    for hh in range(2):
                        h = pair * 2 + hh
                        wei = weip.tile([P, 512], DT_MM, tag="wei")
                        nc.scalar.activation(
                            wei[:, o:512], sc[hh][:, o:512], AF.Exp,
                            scale=float(HS) ** -0.5,
                        )
                        if diag:
                            nc.vector.tensor_tensor(
                                wei[:, ds(o, P)], wei[:, ds(o, P)], tri[:], ALU.mult
                            )
                        nc.tensor.matmul(
                            pvh[hh][:, o:512],
                            _r(vA[:, j, h, :]),
                            _r(wei[:, o:512]),
                            start=(j == 0), stop=(j == jmax),
                        )
                # normalize + write out
                for hh in range(2):
                    h = pair * 2 + hh
                    ot = otp.tile([HS + 1, 512], F32, tag="ot")
                    nc.scalar.copy(ot[:], pvh[hh][:])
                    ptn = ps_nrm.tile([P, 4, HS + 1], F32, tag="nrm")
                    for i in range(4):
                        nc.tensor.matmul(
                            ptn[:, i, :], ot[:, ts(i, P)], ident[0:HS + 1, 0:HS + 1],
                            is_transpose=True, start=(i == 0), stop=(i == 3),
                        )
                    for i in range(4):
                        rc = fin.tile([P, 1], F32, tag="rc")
                        nc.vector.reciprocal(rc[:], ptn[:, i, HS:HS + 1])
                        fo = fin.tile([P, HS], F32, tag="fo")
                        nc.vector.tensor_scalar_mul(fo[:], ptn[:, i, 0:HS], rc[:])
                        nc.sync.dma_start(out_d[ds(qc + i * P, P), ds(h * HS, HS)], fo[:])


_CACHED_NC = None


def _build():
    global _CACHED_NC
    if _CACHED_NC is not None:
        return _CACHED_NC
    nc = bacc.Bacc("TRN2", target_bir_lowering=False, debug=False,
                   num_devices=NCORES)
    xb = nc.dram_tensor("xb", [T, C], F32, kind="ExternalInput").ap()
    w = nc.dram_tensor("w", [C, 3 * GCOLS], F32R if USE_F32R else F32, kind="ExternalInput").ap()
    bvec = nc.dram_tensor("b", [3 * GCOLS], F32, kind="ExternalInput").ap()
    out_d = nc.dram_tensor("out", [T, GCOLS], F32, kind="ExternalOutput").ap()
    with tile.TileContext(nc) as tc:
        _emit(tc, nc, xb, w, bvec, out_d)
    nc.compile()
    _CACHED_NC = nc
    return nc


def _in_maps(x, W_attn, b_attn):
    x = np.asarray(x, dtype=np.float32)
    W = np.asarray(W_attn, dtype=np.float32)
    bias = np.asarray(b_attn, dtype=np.float32)
    maps = []
    for c in range(NCORES):
        b_idx, g = c // 4, c % 4
        cols = slice(g * GCOLS, (g + 1) * GCOLS)
        wc = np.concatenate(
            [W[:, cols], W[:, C:][:, cols], W[:, 2 * C:][:, cols]], axis=1
        )
        bc = np.concatenate(
            [bias[cols], bias[C:][cols], bias[2 * C:][cols]], axis=0
        )
        maps.append({
            "xb": np.ascontiguousarray(x[b_idx]),
            "w": np.ascontiguousarray(wc),
            "b": np.ascontiguousarray(bc),
        })
    return maps


def run(x, W_attn, b_attn, trace=False):
    nc = _build()
    maps = _in_maps(x, W_attn, b_attn)
    res = bass_utils.run_bass_kernel_spmd(
        nc, maps, list(range(NCORES)), trace=trace,
        trace_cores=[0] if trace else None,
    )
    out = np.empty((B, T, C), dtype=np.float32)
    for c in range(NCORES):
        b_idx, g = c // 4, c % 4
        out[b_idx, :, g * GCOLS:(g + 1) * GCOLS] = res.results[c]["out"]
    return out, res


def kernel(x, W_attn, b_attn):
    out, _ = run(x, W_attn, b_attn, trace=False)
    return out
